# revision 25
# baseline (speedup 1.0000x reference)
"""MaxK-GCN conv on 8 Trainium2 NeuronCores.

Pipeline (per core c, SPMD over 8 cores; nodes sharded 8 x 12500):
  phase 1: h = featT_c.T @ W (PE), top-16-of-64 threshold mask (DVE max8 +
           match_replace), scale by (max(out_deg,1)*max(in_deg,1))^-0.5, and
           split each fp32 row into a [hi|lo] bf16 pair -> local table shard
           [12544, 128] bf16 (hi+lo reconstructs fp32 to ~2^-17).
  AllGather table shards -> full table [100352, 128] bf16 in DRAM.
  phase 2: edges with dst in shard c, host-sorted by (pass, slab, block):
           dma_gather src rows (4 SWDGE queues), one-hot S tiles from dst
           values (DVE is_eq vs iota), matmul S^T @ G accumulating per
           128-dst-node block in PSUM (PE), fold hi+lo + bias (DVE), DMA out.

Edge bookkeeping (sort, padding, degree counts) is host-side index metadata;
all floating-point math runs on device.
"""
import sys
import os

sys.path.insert(0, "/opt/trn_rl_repo")

import numpy as np
import ml_dtypes
import concourse.bacc as bacc
import concourse.mybir as mybir
import concourse.tile as tile
from concourse.bass_utils import run_bass_kernel_spmd

P = 128
N_NODES = 100000
IN_FEATS = 256
OUT_FEATS = 64
N_CORES = 8
SHARD = N_NODES // N_CORES          # 12500 real nodes per core
SHARD_PAD = 12544                   # 98 * 128
N_BLOCKS = SHARD_PAD // P           # 98
N_SLABS = 4
QROWS = SHARD_PAD // 4              # 3136 rows per shard-quarter
SLAB_ROWS = N_CORES * QROWS         # 25088 rows per quarter-table
PASS_BLOCKS = [8] * 12 + [2]        # blocks per PSUM pass (1 block = 1 bank)
N_PASSES = len(PASS_BLOCKS)
TABLE_ROWS = N_CORES * SHARD_PAD    # 100352
PAIR = 2 * OUT_FEATS                # 128 bf16 per table row (hi|lo)
TILES_PER_CALL = 32                 # <= 32*128 idx per dma_gather call
NEG_INF = -3.0e38

PASS_OF_BLOCK = np.repeat(np.arange(N_PASSES), PASS_BLOCKS)
PASS_BASE = np.cumsum([0] + PASS_BLOCKS[:-1])


def _inspect(src, dst):
    """Host inspector: per-core sorted edge data + shared static tile grid."""
    src = src.astype(np.int64)
    dst = dst.astype(np.int64)
    core = dst // SHARD
    gidx_of, dstrel_of = [], []
    counts = np.zeros((N_CORES, N_PASSES, N_SLABS, N_BLOCKS), dtype=np.int64)
    for c in range(N_CORES):
        m = core == c
        s_c = src[m]
        d_loc = dst[m] - c * SHARD
        blk = d_loc >> 7
        s8 = s_c // SHARD
        local = s_c % SHARD
        slab = local // QROWS                 # quarter of the owning shard
        gidx = s8 * QROWS + (local - slab * QROWS)  # row within quarter-table
        pss = PASS_OF_BLOCK[blk]
        order = np.lexsort((gidx, blk, slab, pss))
        gidx_of.append(gidx[order])
        dstrel_of.append((d_loc - (blk << 7))[order])
        key = (pss * N_SLABS + slab) * N_BLOCKS + blk
        cnt = np.bincount(key, minlength=N_PASSES * N_SLABS * N_BLOCKS)
        counts[c] = cnt.reshape(N_PASSES, N_SLABS, N_BLOCKS)
    T = ((counts + P - 1) // P).max(axis=0)  # shared tile grid
    return gidx_of, dstrel_of, counts, T


def _make_schedule(T):
    """Static tile stream + gather call list, identical on all cores."""
    tile_meta = []   # [p, s, b, start, stop]
    for p in range(N_PASSES):
        blocks = range(PASS_BASE[p], PASS_BASE[p] + PASS_BLOCKS[p])
        first = {}
        last = {}
        for s in range(N_SLABS):
            for b in blocks:
                for _ in range(T[p, s, b]):
                    j = len(tile_meta)
                    first.setdefault(b, j)
                    last[b] = j
                    tile_meta.append([p, s, b, False, False])
        for b in first:
            tile_meta[first[b]][3] = True
            tile_meta[last[b]][4] = True
    calls = []
    j, n = 0, len(tile_meta)
    while j < n:
        p, s = tile_meta[j][0], tile_meta[j][1]
        k = j
        while k < n and tile_meta[k][0] == p and tile_meta[k][1] == s \
                and k - j < TILES_PER_CALL:
            k += 1
        calls.append((s, j, k - j))
        j = k
    return tile_meta, calls


def _per_core_streams(c, tile_meta, counts, gidx_of, dstrel_of):
    """This core's padded gather-idx + dst_rel streams matching the grid."""
    ntiles = len(tile_meta)
    idx_stream = np.zeros(ntiles * P, dtype=np.int16)
    dst_stream = np.full(ntiles * P, -1.0, dtype=np.float32)
    edge_ptr = 0
    j = 0
    while j < ntiles:
        p, s, b = tile_meta[j][:3]
        k = j
        while k < ntiles and tile_meta[k][:3] == [p, s, b]:
            k += 1
        nseg = int(counts[c, p, s, b])
        base = j * P
        idx_stream[base:base + nseg] = gidx_of[c][edge_ptr:edge_ptr + nseg]
        dst_stream[base:base + nseg] = dstrel_of[c][edge_ptr:edge_ptr + nseg]
        edge_ptr += nseg
        j = k
    assert edge_ptr == len(gidx_of[c])
    idx_wrapped = np.tile(idx_stream.reshape(-1, 16).T, (8, 1)).copy()
    dstv = dst_stream.reshape(ntiles, P).T.copy()
    return idx_wrapped, dstv


def _build(tile_meta, calls):
    ntiles = len(tile_meta)
    nc = bacc.Bacc("TRN2", target_bir_lowering=False, num_swdge_queues=4)
    dt = mybir.dt

    featT = nc.declare_dram_parameter("featT", [IN_FEATS, SHARD_PAD], dt.float32, isOutput=False)
    w_in = nc.declare_dram_parameter("w", [IN_FEATS, OUT_FEATS], dt.float32, isOutput=False)
    biasb = nc.declare_dram_parameter("biasb", [P, OUT_FEATS], dt.float32, isOutput=False)
    idegw = nc.declare_dram_parameter("idegw", [P, N_BLOCKS], dt.float32, isOutput=False)
    odegw = nc.declare_dram_parameter("odegw", [P, N_BLOCKS], dt.float32, isOutput=False)
    iota_in = nc.declare_dram_parameter("iota", [P, P], dt.bfloat16, isOutput=False)
    idxs_in = nc.declare_dram_parameter("idxs", [P, ntiles * 8], dt.int16, isOutput=False)
    dstv_in = nc.declare_dram_parameter("dstv", [P, ntiles], dt.bfloat16, isOutput=False)
    out_d = nc.declare_dram_parameter("out", [SHARD_PAD, OUT_FEATS], dt.float32, isOutput=True)

    tableL = nc.dram_tensor("tableL", [SHARD_PAD, PAIR], dt.bfloat16)
    tableQ = [nc.dram_tensor(f"tableQ{q}", [SLAB_ROWS, PAIR], dt.bfloat16,
                             addr_space="Shared") for q in range(N_SLABS)]

    with tile.TileContext(nc) as tc:
        with tc.tile_pool(name="const", bufs=1) as constp, \
             tc.tile_pool(name="gp", bufs=6) as gp, \
             tc.tile_pool(name="sp", bufs=8) as sps, \
             tc.tile_pool(name="outp", bufs=2) as outp:

            # ---- constants ----
            w_sb = constp.tile([P, 2, OUT_FEATS], dt.float32)
            for k in range(2):
                nc.sync.dma_start(out=w_sb[:, k, :], in_=w_in[k * P:(k + 1) * P, :])
            bias_sb = constp.tile([P, OUT_FEATS], dt.float32)
            nc.sync.dma_start(out=bias_sb[:], in_=biasb[:])
            iota_sb = constp.tile([P, 1, P], dt.bfloat16)
            nc.sync.dma_start(out=iota_sb[:, 0, :], in_=iota_in[:])
            dstv_sb = constp.tile([P, ntiles, 1], dt.bfloat16)
            nc.sync.dma_start(out=dstv_sb[:, :, 0], in_=dstv_in[:])
            idx_sb = constp.tile([P, ntiles * 8], dt.int16)
            nc.sync.dma_start(out=idx_sb[:], in_=idxs_in[:])

            # ---- phase 1: table build (pools scoped to free SBUF/PSUM) ----
            with tc.tile_pool(name="ft", bufs=1) as ftp, \
                 tc.tile_pool(name="ph1", bufs=4) as ph1, \
                 tc.tile_pool(name="ph1ps", bufs=4, space="PSUM") as ph1ps:

                ideg_sb = ph1.tile([P, N_BLOCKS], dt.float32, tag="deg")
                odeg_sb = ph1.tile([P, N_BLOCKS], dt.float32, tag="deg")
                nc.sync.dma_start(out=ideg_sb[:], in_=idegw[:])
                nc.sync.dma_start(out=odeg_sb[:], in_=odegw[:])
                scale_sb = constp.tile([P, N_BLOCKS], dt.float32)
                nc.vector.tensor_scalar_max(ideg_sb[:], ideg_sb[:], 1.0)
                nc.vector.tensor_scalar_max(odeg_sb[:], odeg_sb[:], 1.0)
                nc.vector.tensor_mul(out=scale_sb[:], in0=ideg_sb[:], in1=odeg_sb[:])
                nc.scalar.activation(out=scale_sb[:], in_=scale_sb[:],
                                     func=mybir.ActivationFunctionType.Sqrt)
                nc.vector.reciprocal(out=scale_sb[:], in_=scale_sb[:])

                ft_sb = []
                for k in range(2):
                    t_ = ftp.tile([P, SHARD_PAD], dt.float32, tag=f"ft{k}")
                    nc.sync.dma_start(out=t_[:], in_=featT[k * P:(k + 1) * P, :])
                    ft_sb.append(t_)

                for t in range(N_BLOCKS):
                    hp = ph1ps.tile([P, OUT_FEATS], dt.float32, tag="hps")
                    for k in range(2):
                        nc.tensor.matmul(
                            out=hp[:],
                            lhsT=ft_sb[k][:, t * P:(t + 1) * P],
                            rhs=w_sb[:, k, :],
                            start=(k == 0), stop=(k == 1),
                        )
                    h = ph1.tile([P, OUT_FEATS], dt.float32, tag="h")
                    nc.vector.tensor_copy(out=h[:], in_=hp[:])
                    m1 = ph1.tile([P, 8], dt.float32, tag="m1")
                    nc.vector.max(m1[:], h[:])
                    hneg = ph1.tile([P, OUT_FEATS], dt.float32, tag="hneg")
                    nc.vector.match_replace(out=hneg[:], in_to_replace=m1[:],
                                            in_values=h[:], imm_value=NEG_INF)
                    m2 = ph1.tile([P, 8], dt.float32, tag="m2")
                    nc.vector.max(m2[:], hneg[:])
                    mask = ph1.tile([P, OUT_FEATS], dt.float32, tag="mask")
                    nc.vector.tensor_tensor(
                        out=mask[:], in0=h[:],
                        in1=m2[:, 7:8].to_broadcast([P, OUT_FEATS]),
                        op=mybir.AluOpType.is_ge)
                    nc.vector.tensor_mul(out=h[:], in0=h[:], in1=mask[:])
                    nc.vector.tensor_tensor(
                        out=h[:], in0=h[:],
                        in1=scale_sb[:, t:t + 1].to_broadcast([P, OUT_FEATS]),
                        op=mybir.AluOpType.mult)
                    ttile = ph1.tile([P, PAIR], dt.bfloat16, tag="ttile")
                    hi32 = ph1.tile([P, OUT_FEATS], dt.float32, tag="hi32")
                    nc.scalar.activation(out=ttile[:, 0:OUT_FEATS], in_=h[:],
                                         func=mybir.ActivationFunctionType.Copy)
                    nc.scalar.activation(out=hi32[:], in_=ttile[:, 0:OUT_FEATS],
                                         func=mybir.ActivationFunctionType.Copy)
                    nc.vector.tensor_sub(out=hi32[:], in0=h[:], in1=hi32[:])
                    nc.vector.tensor_copy(out=ttile[:, OUT_FEATS:PAIR], in_=hi32[:])
                    nc.sync.dma_start(out=tableL[t * P:(t + 1) * P, :], in_=ttile[:])

            # ---- allgather table, one collective per shard-quarter so
            # phase-2 gathers can start before phase 1 fully drains ----
            for q in range(N_SLABS):
                nc.gpsimd.collective_compute(
                    "AllGather",
                    mybir.AluOpType.bypass,
                    replica_groups=[list(range(N_CORES))],
                    ins=[tableL[q * QROWS:(q + 1) * QROWS, :]],
                    outs=[tableQ[q][:]],
                )

            # ---- phase 2: edge aggregation ----
            phase2_stack = __import__("contextlib").ExitStack()
            accp = phase2_stack.enter_context(
                tc.tile_pool(name="accp", bufs=1, space="PSUM"))
            g_tiles = {}
            for ci, (s, j0, ct) in enumerate(calls):
                g = gp.tile([P, TILES_PER_CALL, PAIR], dt.bfloat16, tag="g")
                nc.gpsimd.dma_gather(
                    out_ap=g[:, :ct, :],
                    in_ap=tableQ[s][:],
                    idxs_ap=idx_sb[:, j0 * 8:(j0 + ct) * 8],
                    num_idxs=ct * P,
                    num_idxs_reg=ct * P,
                    elem_size=PAIR,
                    single_packet=False,
                    queue_num=ci % 4,
                )
                for t in range(ct):
                    g_tiles[j0 + t] = (g, t)

            # batched one-hot builds: one DVE op per 4 tiles
            SW = 4
            s_tiles = {}
            for j0 in range(0, ntiles, SW):
                jn = min(SW, ntiles - j0)
                s4 = sps.tile([P, SW, P], dt.bfloat16, tag="s")
                nc.vector.tensor_tensor(
                    out=s4[:, :jn, :],
                    in0=dstv_sb[:, j0:j0 + jn, :].to_broadcast([P, jn, P]),
                    in1=iota_sb[:].to_broadcast([P, jn, P]),
                    op=mybir.AluOpType.is_equal)
                for t in range(jn):
                    s_tiles[j0 + t] = (s4, t)

            cur_pass = -1
            acc = None
            ostage = None
            for j, (p, s, b, st, sp_) in enumerate(tile_meta):
                if p != cur_pass:
                    # one 2KB bank (512 fp32) per block so each block's
                    # matmul accumulation group owns its bank
                    acc = accp.tile([P, max(PASS_BLOCKS), 512], dt.float32,
                                    tag="acc")
                    ostage = outp.tile([P, max(PASS_BLOCKS), OUT_FEATS],
                                       dt.float32, tag="ostage")
                    cur_pass = p
                b_rel = b - PASS_BASE[p]
                s4_t, s4_i = s_tiles[j]
                s_t = s4_t[:, s4_i, :]
                g, gt = g_tiles[j]
                nc.tensor.matmul(
                    out=acc[:, b_rel, 0:PAIR],
                    lhsT=s_t[:],
                    rhs=g[:, gt, :],
                    start=bool(st), stop=bool(sp_),
                    skip_group_check=True,
                )
                if sp_:
                    nc.vector.tensor_add(out=ostage[:, b_rel, :],
                                         in0=acc[:, b_rel, 0:OUT_FEATS],
                                         in1=bias_sb[:])
                    nc.vector.tensor_add(out=ostage[:, b_rel, :],
                                         in0=ostage[:, b_rel, :],
                                         in1=acc[:, b_rel, OUT_FEATS:PAIR])
                    nc.sync.dma_start(out=out_d[b * P:(b + 1) * P, :],
                                      in_=ostage[:, b_rel, :])
            phase2_stack.close()

    nc.finalize()
    return nc


def kernel(feat, weight, bias, src, dst):
    feat = np.asarray(feat, dtype=np.float32)
    weight = np.asarray(weight, dtype=np.float32)
    bias = np.asarray(bias, dtype=np.float32)
    src = np.asarray(src)
    dst = np.asarray(dst)

    gidx_of, dstrel_of, counts, T = _inspect(src, dst)
    tile_meta, calls = _make_schedule(T)

    in_deg = np.bincount(dst, minlength=N_NODES).astype(np.float32)
    out_deg = np.bincount(src, minlength=N_NODES).astype(np.float32)
    ft = feat.T  # [256, 100000]
    iota = np.tile(np.arange(P, dtype=np.float32), (P, 1)).astype(ml_dtypes.bfloat16)

    in_maps = []
    for c in range(N_CORES):
        lo, hi = c * SHARD, (c + 1) * SHARD
        featT_c = np.zeros((IN_FEATS, SHARD_PAD), dtype=np.float32)
        featT_c[:, :SHARD] = ft[:, lo:hi]
        ideg_c = np.ones(SHARD_PAD, dtype=np.float32)
        odeg_c = np.ones(SHARD_PAD, dtype=np.float32)
        ideg_c[:SHARD] = in_deg[lo:hi]
        odeg_c[:SHARD] = out_deg[lo:hi]
        idx_wrapped, dstv = _per_core_streams(c, tile_meta, counts,
                                              gidx_of, dstrel_of)
        in_maps.append({
            "featT": featT_c,
            "w": weight,
            "biasb": np.tile(bias[None, :], (P, 1)).astype(np.float32),
            "idegw": ideg_c.reshape(N_BLOCKS, P).T.copy(),
            "odegw": odeg_c.reshape(N_BLOCKS, P).T.copy(),
            "iota": iota,
            "idxs": idx_wrapped,
            "dstv": dstv.astype(ml_dtypes.bfloat16),
        })

    nc = _build(tile_meta, calls)
    res = run_bass_kernel_spmd(nc, in_maps, list(range(N_CORES)),
                               trace=bool(os.environ.get("KERNEL_TRACE")))
    if os.environ.get("KERNEL_TRACE"):
        print(f"HW exec time: {res.exec_time_ns} ns")
    out = np.concatenate(
        [res.results[c]["out"][:SHARD] for c in range(N_CORES)], axis=0)
    return out.astype(np.float32)


# revision 26
# speedup vs baseline: 1.2130x; 1.2130x over previous
"""MaxK-GCN conv on 8 Trainium2 NeuronCores.

Pipeline (per core c, SPMD over 8 cores; nodes sharded 8 x 12500):
  phase 1: h = featT_c.T @ W (PE), top-16-of-64 threshold mask (DVE max8 +
           match_replace), scale by (max(out_deg,1)*max(in_deg,1))^-0.5, and
           split each fp32 row into a [hi|lo] bf16 pair -> local table shard
           [12544, 128] bf16 (hi+lo reconstructs fp32 to ~2^-17).
  AllGather table shards -> full table [100352, 128] bf16 in DRAM.
  phase 2: edges with dst in shard c, host-sorted by (pass, slab, block):
           dma_gather src rows (4 SWDGE queues), one-hot S tiles from dst
           values (DVE is_eq vs iota), matmul S^T @ G accumulating per
           128-dst-node block in PSUM (PE), fold hi+lo + bias (DVE), DMA out.

Edge bookkeeping (sort, padding, degree counts) is host-side index metadata;
all floating-point math runs on device.
"""
import sys
import os

sys.path.insert(0, "/opt/trn_rl_repo")

import numpy as np
import ml_dtypes
import concourse.bacc as bacc
import concourse.mybir as mybir
import concourse.tile as tile
from concourse.bass_utils import run_bass_kernel_spmd

P = 128
N_NODES = 100000
IN_FEATS = 256
OUT_FEATS = 64
N_CORES = 8
SHARD = N_NODES // N_CORES          # 12500 real nodes per core
SHARD_PAD = 12544                   # 98 * 128
N_BLOCKS = SHARD_PAD // P           # 98
N_SLABS = 4
QROWS = SHARD_PAD // 4              # 3136 rows per shard-quarter
SLAB_ROWS = N_CORES * QROWS         # 25088 rows per quarter-table
PASS_BLOCKS = [8] * 12 + [2]        # blocks per PSUM pass (1 block = 1 bank)
N_PASSES = len(PASS_BLOCKS)
TABLE_ROWS = N_CORES * SHARD_PAD    # 100352
PAIR = 2 * OUT_FEATS                # 128 bf16 per table row (hi|lo)
TILES_PER_CALL = 24                 # <= 24*128 idx per dma_gather call
NEG_INF = -3.0e38

PASS_OF_BLOCK = np.repeat(np.arange(N_PASSES), PASS_BLOCKS)
PASS_BASE = np.cumsum([0] + PASS_BLOCKS[:-1])


def _inspect(src, dst):
    """Host inspector: per-core sorted edge data + shared static tile grid."""
    src = src.astype(np.int64)
    dst = dst.astype(np.int64)
    core = dst // SHARD
    gidx_of, dstrel_of = [], []
    counts = np.zeros((N_CORES, N_PASSES, N_SLABS, N_BLOCKS), dtype=np.int64)
    for c in range(N_CORES):
        m = core == c
        s_c = src[m]
        d_loc = dst[m] - c * SHARD
        blk = d_loc >> 7
        s8 = s_c // SHARD
        local = s_c % SHARD
        slab = local // QROWS                 # quarter of the owning shard
        gidx = s8 * QROWS + (local - slab * QROWS)  # row within quarter-table
        pss = PASS_OF_BLOCK[blk]
        order = np.lexsort((gidx, blk, slab, pss))
        gidx_of.append(gidx[order])
        dstrel_of.append((d_loc - (blk << 7))[order])
        key = (pss * N_SLABS + slab) * N_BLOCKS + blk
        cnt = np.bincount(key, minlength=N_PASSES * N_SLABS * N_BLOCKS)
        counts[c] = cnt.reshape(N_PASSES, N_SLABS, N_BLOCKS)
    T = ((counts + P - 1) // P).max(axis=0)  # shared tile grid
    return gidx_of, dstrel_of, counts, T


def _make_schedule(T):
    """Static tile stream + gather call list, identical on all cores."""
    tile_meta = []   # [p, s, b, start, stop]
    for p in range(N_PASSES):
        blocks = range(PASS_BASE[p], PASS_BASE[p] + PASS_BLOCKS[p])
        first = {}
        last = {}
        for s in range(N_SLABS):
            for b in blocks:
                for _ in range(T[p, s, b]):
                    j = len(tile_meta)
                    first.setdefault(b, j)
                    last[b] = j
                    tile_meta.append([p, s, b, False, False])
        for b in first:
            tile_meta[first[b]][3] = True
            tile_meta[last[b]][4] = True
    calls = []
    j, n = 0, len(tile_meta)
    while j < n:
        p, s = tile_meta[j][0], tile_meta[j][1]
        k = j
        while k < n and tile_meta[k][0] == p and tile_meta[k][1] == s \
                and k - j < TILES_PER_CALL:
            k += 1
        calls.append((s, j, k - j))
        j = k
    return tile_meta, calls


def _per_core_streams(c, tile_meta, counts, gidx_of, dstrel_of):
    """This core's padded gather-idx + dst_rel streams matching the grid."""
    ntiles = len(tile_meta)
    idx_stream = np.zeros(ntiles * P, dtype=np.int16)
    dst_stream = np.full(ntiles * P, -1.0, dtype=np.float32)
    edge_ptr = 0
    j = 0
    while j < ntiles:
        p, s, b = tile_meta[j][:3]
        k = j
        while k < ntiles and tile_meta[k][:3] == [p, s, b]:
            k += 1
        nseg = int(counts[c, p, s, b])
        base = j * P
        idx_stream[base:base + nseg] = gidx_of[c][edge_ptr:edge_ptr + nseg]
        dst_stream[base:base + nseg] = dstrel_of[c][edge_ptr:edge_ptr + nseg]
        edge_ptr += nseg
        j = k
    assert edge_ptr == len(gidx_of[c])
    idx_wrapped = np.tile(idx_stream.reshape(-1, 16).T, (8, 1)).copy()
    dstv = dst_stream.reshape(ntiles, P).T.copy()
    return idx_wrapped, dstv


def _build(tile_meta, calls):
    ntiles = len(tile_meta)
    nc = bacc.Bacc("TRN2", target_bir_lowering=False, num_swdge_queues=4)
    dt = mybir.dt

    featT = nc.declare_dram_parameter("featT", [IN_FEATS, SHARD_PAD], dt.float32, isOutput=False)
    w_in = nc.declare_dram_parameter("w", [IN_FEATS, OUT_FEATS], dt.float32, isOutput=False)
    biasb = nc.declare_dram_parameter("biasb", [P, OUT_FEATS], dt.float32, isOutput=False)
    idegw = nc.declare_dram_parameter("idegw", [P, N_BLOCKS], dt.float32, isOutput=False)
    odegw = nc.declare_dram_parameter("odegw", [P, N_BLOCKS], dt.float32, isOutput=False)
    iota_in = nc.declare_dram_parameter("iota", [P, P], dt.bfloat16, isOutput=False)
    idxs_in = nc.declare_dram_parameter("idxs", [P, ntiles * 8], dt.int16, isOutput=False)
    dstv_in = nc.declare_dram_parameter("dstv", [P, ntiles], dt.bfloat16, isOutput=False)
    out_d = nc.declare_dram_parameter("out", [SHARD_PAD, OUT_FEATS], dt.float32, isOutput=True)

    tableL = nc.dram_tensor("tableL", [SHARD_PAD, PAIR], dt.bfloat16)
    tableQ = [nc.dram_tensor(f"tableQ{q}", [SLAB_ROWS, PAIR], dt.bfloat16,
                             addr_space="Shared") for q in range(N_SLABS)]

    with tile.TileContext(nc) as tc:
        with tc.tile_pool(name="const", bufs=1) as constp, \
             tc.tile_pool(name="gp", bufs=6) as gp, \
             tc.tile_pool(name="sp", bufs=8) as sps, \
             tc.tile_pool(name="outp", bufs=2) as outp:

            # ---- constants ----
            w_sb = constp.tile([P, 2, OUT_FEATS], dt.float32)
            for k in range(2):
                nc.sync.dma_start(out=w_sb[:, k, :], in_=w_in[k * P:(k + 1) * P, :])
            bias_sb = constp.tile([P, OUT_FEATS], dt.float32)
            nc.sync.dma_start(out=bias_sb[:], in_=biasb[:])
            iota_sb = constp.tile([P, 1, P], dt.bfloat16)
            nc.sync.dma_start(out=iota_sb[:, 0, :], in_=iota_in[:])
            dstv_sb = constp.tile([P, ntiles, 1], dt.bfloat16)
            nc.sync.dma_start(out=dstv_sb[:, :, 0], in_=dstv_in[:])
            idx_sb = constp.tile([P, ntiles * 8], dt.int16)
            nc.sync.dma_start(out=idx_sb[:], in_=idxs_in[:])

            # ---- phase 1: table build (pools scoped to free SBUF/PSUM) ----
            with tc.tile_pool(name="ft", bufs=1) as ftp, \
                 tc.tile_pool(name="ph1", bufs=4) as ph1, \
                 tc.tile_pool(name="ph1ps", bufs=4, space="PSUM") as ph1ps:

                ideg_sb = ph1.tile([P, N_BLOCKS], dt.float32, tag="deg")
                odeg_sb = ph1.tile([P, N_BLOCKS], dt.float32, tag="deg")
                nc.sync.dma_start(out=ideg_sb[:], in_=idegw[:])
                nc.sync.dma_start(out=odeg_sb[:], in_=odegw[:])
                scale_sb = constp.tile([P, N_BLOCKS], dt.float32)
                nc.vector.tensor_scalar_max(ideg_sb[:], ideg_sb[:], 1.0)
                nc.vector.tensor_scalar_max(odeg_sb[:], odeg_sb[:], 1.0)
                nc.vector.tensor_mul(out=scale_sb[:], in0=ideg_sb[:], in1=odeg_sb[:])
                nc.scalar.activation(out=scale_sb[:], in_=scale_sb[:],
                                     func=mybir.ActivationFunctionType.Sqrt)
                nc.vector.reciprocal(out=scale_sb[:], in_=scale_sb[:])

                ft_sb = []
                for k in range(2):
                    t_ = ftp.tile([P, SHARD_PAD], dt.float32, tag=f"ft{k}")
                    nc.sync.dma_start(out=t_[:], in_=featT[k * P:(k + 1) * P, :])
                    ft_sb.append(t_)

                for t in range(N_BLOCKS):
                    hp = ph1ps.tile([P, OUT_FEATS], dt.float32, tag="hps")
                    for k in range(2):
                        nc.tensor.matmul(
                            out=hp[:],
                            lhsT=ft_sb[k][:, t * P:(t + 1) * P],
                            rhs=w_sb[:, k, :],
                            start=(k == 0), stop=(k == 1),
                        )
                    h = ph1.tile([P, OUT_FEATS], dt.float32, tag="h")
                    nc.vector.tensor_copy(out=h[:], in_=hp[:])
                    m1 = ph1.tile([P, 8], dt.float32, tag="m1")
                    nc.vector.max(m1[:], h[:])
                    hneg = ph1.tile([P, OUT_FEATS], dt.float32, tag="hneg")
                    nc.vector.match_replace(out=hneg[:], in_to_replace=m1[:],
                                            in_values=h[:], imm_value=NEG_INF)
                    m2 = ph1.tile([P, 8], dt.float32, tag="m2")
                    nc.vector.max(m2[:], hneg[:])
                    mask = ph1.tile([P, OUT_FEATS], dt.float32, tag="mask")
                    nc.vector.tensor_tensor(
                        out=mask[:], in0=h[:],
                        in1=m2[:, 7:8].to_broadcast([P, OUT_FEATS]),
                        op=mybir.AluOpType.is_ge)
                    nc.vector.tensor_mul(out=h[:], in0=h[:], in1=mask[:])
                    nc.vector.tensor_tensor(
                        out=h[:], in0=h[:],
                        in1=scale_sb[:, t:t + 1].to_broadcast([P, OUT_FEATS]),
                        op=mybir.AluOpType.mult)
                    ttile = ph1.tile([P, PAIR], dt.bfloat16, tag="ttile")
                    hi32 = ph1.tile([P, OUT_FEATS], dt.float32, tag="hi32")
                    nc.scalar.activation(out=ttile[:, 0:OUT_FEATS], in_=h[:],
                                         func=mybir.ActivationFunctionType.Copy)
                    nc.scalar.activation(out=hi32[:], in_=ttile[:, 0:OUT_FEATS],
                                         func=mybir.ActivationFunctionType.Copy)
                    nc.vector.tensor_sub(out=hi32[:], in0=h[:], in1=hi32[:])
                    nc.vector.tensor_copy(out=ttile[:, OUT_FEATS:PAIR], in_=hi32[:])
                    nc.sync.dma_start(out=tableL[t * P:(t + 1) * P, :], in_=ttile[:])

            # ---- allgather table, one collective per shard-quarter so
            # phase-2 gathers can start before phase 1 fully drains ----
            for q in range(N_SLABS):
                nc.gpsimd.collective_compute(
                    "AllGather",
                    mybir.AluOpType.bypass,
                    replica_groups=[list(range(N_CORES))],
                    ins=[tableL[q * QROWS:(q + 1) * QROWS, :]],
                    outs=[tableQ[q][:]],
                )

            # ---- phase 2: edge aggregation ----
            phase2_stack = __import__("contextlib").ExitStack()
            accp = phase2_stack.enter_context(
                tc.tile_pool(name="accp", bufs=1, space="PSUM"))
            g_tiles = {}
            for ci, (s, j0, ct) in enumerate(calls):
                g = gp.tile([P, TILES_PER_CALL, PAIR], dt.bfloat16, tag="g")
                nc.gpsimd.dma_gather(
                    out_ap=g[:, :ct, :],
                    in_ap=tableQ[s][:],
                    idxs_ap=idx_sb[:, j0 * 8:(j0 + ct) * 8],
                    num_idxs=ct * P,
                    num_idxs_reg=ct * P,
                    elem_size=PAIR,
                    single_packet=False,
                    queue_num=ci % 4,
                )
                for t in range(ct):
                    g_tiles[j0 + t] = (g, t)

            # batched one-hot builds: one DVE op per 4 tiles
            SW = 4
            s_tiles = {}
            for j0 in range(0, ntiles, SW):
                jn = min(SW, ntiles - j0)
                s4 = sps.tile([P, SW, P], dt.bfloat16, tag="s")
                nc.vector.tensor_tensor(
                    out=s4[:, :jn, :],
                    in0=dstv_sb[:, j0:j0 + jn, :].to_broadcast([P, jn, P]),
                    in1=iota_sb[:].to_broadcast([P, jn, P]),
                    op=mybir.AluOpType.is_equal)
                for t in range(jn):
                    s_tiles[j0 + t] = (s4, t)

            cur_pass = -1
            acc = None
            ostage = None
            for j, (p, s, b, st, sp_) in enumerate(tile_meta):
                if p != cur_pass:
                    # one 2KB bank (512 fp32) per block so each block's
                    # matmul accumulation group owns its bank
                    acc = accp.tile([P, max(PASS_BLOCKS), 512], dt.float32,
                                    tag="acc")
                    ostage = outp.tile([P, max(PASS_BLOCKS), OUT_FEATS],
                                       dt.float32, tag="ostage")
                    cur_pass = p
                b_rel = b - PASS_BASE[p]
                s4_t, s4_i = s_tiles[j]
                s_t = s4_t[:, s4_i, :]
                g, gt = g_tiles[j]
                nc.tensor.matmul(
                    out=acc[:, b_rel, 0:PAIR],
                    lhsT=s_t[:],
                    rhs=g[:, gt, :],
                    start=bool(st), stop=bool(sp_),
                    skip_group_check=True,
                )
                if sp_:
                    nc.vector.tensor_add(out=ostage[:, b_rel, :],
                                         in0=acc[:, b_rel, 0:OUT_FEATS],
                                         in1=bias_sb[:])
                    nc.vector.tensor_add(out=ostage[:, b_rel, :],
                                         in0=ostage[:, b_rel, :],
                                         in1=acc[:, b_rel, OUT_FEATS:PAIR])
                    nc.sync.dma_start(out=out_d[b * P:(b + 1) * P, :],
                                      in_=ostage[:, b_rel, :])
            phase2_stack.close()

    nc.finalize()
    return nc


def kernel(feat, weight, bias, src, dst):
    feat = np.asarray(feat, dtype=np.float32)
    weight = np.asarray(weight, dtype=np.float32)
    bias = np.asarray(bias, dtype=np.float32)
    src = np.asarray(src)
    dst = np.asarray(dst)

    gidx_of, dstrel_of, counts, T = _inspect(src, dst)
    tile_meta, calls = _make_schedule(T)

    in_deg = np.bincount(dst, minlength=N_NODES).astype(np.float32)
    out_deg = np.bincount(src, minlength=N_NODES).astype(np.float32)
    ft = feat.T  # [256, 100000]
    iota = np.tile(np.arange(P, dtype=np.float32), (P, 1)).astype(ml_dtypes.bfloat16)

    in_maps = []
    for c in range(N_CORES):
        lo, hi = c * SHARD, (c + 1) * SHARD
        featT_c = np.zeros((IN_FEATS, SHARD_PAD), dtype=np.float32)
        featT_c[:, :SHARD] = ft[:, lo:hi]
        ideg_c = np.ones(SHARD_PAD, dtype=np.float32)
        odeg_c = np.ones(SHARD_PAD, dtype=np.float32)
        ideg_c[:SHARD] = in_deg[lo:hi]
        odeg_c[:SHARD] = out_deg[lo:hi]
        idx_wrapped, dstv = _per_core_streams(c, tile_meta, counts,
                                              gidx_of, dstrel_of)
        in_maps.append({
            "featT": featT_c,
            "w": weight,
            "biasb": np.tile(bias[None, :], (P, 1)).astype(np.float32),
            "idegw": ideg_c.reshape(N_BLOCKS, P).T.copy(),
            "odegw": odeg_c.reshape(N_BLOCKS, P).T.copy(),
            "iota": iota,
            "idxs": idx_wrapped,
            "dstv": dstv.astype(ml_dtypes.bfloat16),
        })

    nc = _build(tile_meta, calls)
    res = run_bass_kernel_spmd(nc, in_maps, list(range(N_CORES)),
                               trace=bool(os.environ.get("KERNEL_TRACE")))
    if os.environ.get("KERNEL_TRACE"):
        print(f"HW exec time: {res.exec_time_ns} ns")
    out = np.concatenate(
        [res.results[c]["out"][:SHARD] for c in range(N_CORES)], axis=0)
    return out.astype(np.float32)


# revision 28
# speedup vs baseline: 1.2478x; 1.0287x over previous
"""MaxK-GCN conv on 8 Trainium2 NeuronCores.

Pipeline (per core c, SPMD over 8 cores; nodes sharded 8 x 12500):
  phase 1: h = featT_c.T @ W (PE), top-16-of-64 threshold mask (DVE max8 +
           match_replace), scale by (max(out_deg,1)*max(in_deg,1))^-0.5, and
           split each fp32 row into a [hi|lo] bf16 pair -> local table shard
           [12544, 128] bf16 (hi+lo reconstructs fp32 to ~2^-17).
  AllGather table shards -> full table [100352, 128] bf16 in DRAM.
  phase 2: edges with dst in shard c, host-sorted by (pass, slab, block):
           dma_gather src rows (4 SWDGE queues), one-hot S tiles from dst
           values (DVE is_eq vs iota), matmul S^T @ G accumulating per
           128-dst-node block in PSUM (PE), fold hi+lo + bias (DVE), DMA out.

Edge bookkeeping (sort, padding, degree counts) is host-side index metadata;
all floating-point math runs on device.
"""
import sys
import os

sys.path.insert(0, "/opt/trn_rl_repo")

import numpy as np
import ml_dtypes
import concourse.bacc as bacc
import concourse.mybir as mybir
import concourse.tile as tile
from concourse.bass_utils import run_bass_kernel_spmd

P = 128
N_NODES = 100000
IN_FEATS = 256
OUT_FEATS = 64
N_CORES = 8
SHARD = N_NODES // N_CORES          # 12500 real nodes per core
SHARD_PAD = 12544                   # 98 * 128
N_BLOCKS = SHARD_PAD // P           # 98
N_SLABS = 4
QROWS = SHARD_PAD // 4              # 3136 rows per shard-quarter
SLAB_ROWS = N_CORES * QROWS         # 25088 rows per quarter-table
PASS_BLOCKS = [8] * 12 + [2]        # blocks per PSUM pass (1 block = 1 bank)
N_PASSES = len(PASS_BLOCKS)
TABLE_ROWS = N_CORES * SHARD_PAD    # 100352
PAIR = 2 * OUT_FEATS                # 128 bf16 per table row (hi|lo)
TILES_PER_CALL = 24                 # <= 24*128 idx per dma_gather call
NEG_INF = -3.0e38

PASS_OF_BLOCK = np.repeat(np.arange(N_PASSES), PASS_BLOCKS)
PASS_BASE = np.cumsum([0] + PASS_BLOCKS[:-1])


def _inspect(src, dst):
    """Host inspector: per-core sorted edge data + shared static tile grid."""
    src = src.astype(np.int64)
    dst = dst.astype(np.int64)
    core = dst // SHARD
    gidx_of, dstrel_of = [], []
    counts = np.zeros((N_CORES, N_PASSES, N_SLABS, N_BLOCKS), dtype=np.int64)
    for c in range(N_CORES):
        m = core == c
        s_c = src[m]
        d_loc = dst[m] - c * SHARD
        blk = d_loc >> 7
        s8 = s_c // SHARD
        local = s_c % SHARD
        slab = local // QROWS                 # quarter of the owning shard
        gidx = s8 * QROWS + (local - slab * QROWS)  # row within quarter-table
        pss = PASS_OF_BLOCK[blk]
        order = np.lexsort((gidx, blk, slab, pss))
        gidx_of.append(gidx[order])
        dstrel_of.append((d_loc - (blk << 7))[order])
        key = (pss * N_SLABS + slab) * N_BLOCKS + blk
        cnt = np.bincount(key, minlength=N_PASSES * N_SLABS * N_BLOCKS)
        counts[c] = cnt.reshape(N_PASSES, N_SLABS, N_BLOCKS)
    T = ((counts + P - 1) // P).max(axis=0)  # shared tile grid
    return gidx_of, dstrel_of, counts, T


def _make_schedule(T):
    """Static tile stream + gather call list, identical on all cores."""
    tile_meta = []   # [p, s, b, start, stop]
    for p in range(N_PASSES):
        blocks = range(PASS_BASE[p], PASS_BASE[p] + PASS_BLOCKS[p])
        first = {}
        last = {}
        for s in range(N_SLABS):
            for b in blocks:
                for _ in range(T[p, s, b]):
                    j = len(tile_meta)
                    first.setdefault(b, j)
                    last[b] = j
                    tile_meta.append([p, s, b, False, False])
        for b in first:
            tile_meta[first[b]][3] = True
            tile_meta[last[b]][4] = True
    calls = []
    j, n = 0, len(tile_meta)
    while j < n:
        p, s = tile_meta[j][0], tile_meta[j][1]
        k = j
        while k < n and tile_meta[k][0] == p and tile_meta[k][1] == s \
                and k - j < TILES_PER_CALL:
            k += 1
        calls.append((s, j, k - j))
        j = k
    return tile_meta, calls


def _per_core_streams(c, tile_meta, counts, gidx_of, dstrel_of):
    """This core's padded gather-idx + dst_rel streams matching the grid."""
    ntiles = len(tile_meta)
    idx_stream = np.zeros(ntiles * P, dtype=np.int16)
    dst_stream = np.full(ntiles * P, -1.0, dtype=np.float32)
    edge_ptr = 0
    j = 0
    while j < ntiles:
        p, s, b = tile_meta[j][:3]
        k = j
        while k < ntiles and tile_meta[k][:3] == [p, s, b]:
            k += 1
        nseg = int(counts[c, p, s, b])
        base = j * P
        idx_stream[base:base + nseg] = gidx_of[c][edge_ptr:edge_ptr + nseg]
        dst_stream[base:base + nseg] = dstrel_of[c][edge_ptr:edge_ptr + nseg]
        edge_ptr += nseg
        j = k
    assert edge_ptr == len(gidx_of[c])
    idx_wrapped = np.tile(idx_stream.reshape(-1, 16).T, (8, 1)).copy()
    dstv = dst_stream.reshape(ntiles, P).T.copy()
    return idx_wrapped, dstv


def _build(tile_meta, calls):
    ntiles = len(tile_meta)
    nc = bacc.Bacc("TRN2", target_bir_lowering=False, num_swdge_queues=4)
    dt = mybir.dt

    featT = nc.declare_dram_parameter("featT", [IN_FEATS, SHARD_PAD], dt.float32, isOutput=False)
    w_in = nc.declare_dram_parameter("w", [IN_FEATS, OUT_FEATS], dt.float32, isOutput=False)
    biasb = nc.declare_dram_parameter("biasb", [P, OUT_FEATS], dt.float32, isOutput=False)
    idegw = nc.declare_dram_parameter("idegw", [P, N_BLOCKS], dt.float32, isOutput=False)
    odegw = nc.declare_dram_parameter("odegw", [P, N_BLOCKS], dt.float32, isOutput=False)
    iota_in = nc.declare_dram_parameter("iota", [P, P], dt.bfloat16, isOutput=False)
    idxs_in = nc.declare_dram_parameter("idxs", [P, ntiles * 8], dt.int16, isOutput=False)
    dstv_in = nc.declare_dram_parameter("dstv", [P, ntiles], dt.bfloat16, isOutput=False)
    out_d = nc.declare_dram_parameter("out", [SHARD_PAD, OUT_FEATS], dt.float32, isOutput=True)

    tableL = nc.dram_tensor("tableL", [SHARD_PAD, PAIR], dt.bfloat16)
    tableQ = [nc.dram_tensor(f"tableQ{q}", [SLAB_ROWS, PAIR], dt.bfloat16,
                             addr_space="Shared") for q in range(N_SLABS)]

    with tile.TileContext(nc) as tc:
        with tc.tile_pool(name="const", bufs=1) as constp, \
             tc.tile_pool(name="gp", bufs=8) as gp, \
             tc.tile_pool(name="sp", bufs=8) as sps, \
             tc.tile_pool(name="outp", bufs=2) as outp:

            # ---- constants ----
            w_sb = constp.tile([P, 2, OUT_FEATS], dt.float32)
            for k in range(2):
                nc.sync.dma_start(out=w_sb[:, k, :], in_=w_in[k * P:(k + 1) * P, :])
            bias_sb = constp.tile([P, OUT_FEATS], dt.float32)
            nc.sync.dma_start(out=bias_sb[:], in_=biasb[:])
            iota_sb = constp.tile([P, 1, P], dt.bfloat16)
            nc.sync.dma_start(out=iota_sb[:, 0, :], in_=iota_in[:])
            dstv_sb = constp.tile([P, ntiles, 1], dt.bfloat16)
            nc.sync.dma_start(out=dstv_sb[:, :, 0], in_=dstv_in[:])
            idx_sb = constp.tile([P, ntiles * 8], dt.int16)
            nc.sync.dma_start(out=idx_sb[:], in_=idxs_in[:])

            # ---- phase 1: table build (pools scoped to free SBUF/PSUM) ----
            with tc.tile_pool(name="ft", bufs=1) as ftp, \
                 tc.tile_pool(name="ph1", bufs=4) as ph1, \
                 tc.tile_pool(name="ph1ps", bufs=4, space="PSUM") as ph1ps:

                ideg_sb = ph1.tile([P, N_BLOCKS], dt.float32, tag="deg")
                odeg_sb = ph1.tile([P, N_BLOCKS], dt.float32, tag="deg")
                nc.sync.dma_start(out=ideg_sb[:], in_=idegw[:])
                nc.sync.dma_start(out=odeg_sb[:], in_=odegw[:])
                scale_sb = constp.tile([P, N_BLOCKS], dt.float32)
                nc.vector.tensor_scalar_max(ideg_sb[:], ideg_sb[:], 1.0)
                nc.vector.tensor_scalar_max(odeg_sb[:], odeg_sb[:], 1.0)
                nc.vector.tensor_mul(out=scale_sb[:], in0=ideg_sb[:], in1=odeg_sb[:])
                nc.scalar.activation(out=scale_sb[:], in_=scale_sb[:],
                                     func=mybir.ActivationFunctionType.Sqrt)
                nc.vector.reciprocal(out=scale_sb[:], in_=scale_sb[:])

                # featT in 25-tile chunks (2 k-chunks x 4 column chunks)
                FCH = [25, 25, 25, 23]
                FBASE = [0, 25, 50, 75]
                ft_sb = {}
                for fc in range(4):
                    for k in range(2):
                        t_ = ftp.tile([P, FCH[fc] * P], dt.float32, tag=f"ft{k}", bufs=2)
                        nc.sync.dma_start(
                            out=t_[:],
                            in_=featT[k * P:(k + 1) * P,
                                      FBASE[fc] * P:(FBASE[fc] + FCH[fc]) * P])
                        ft_sb[(fc, k)] = t_

                for t in range(N_BLOCKS):
                    fc = min(t // 25, 3)
                    tc_rel = t - FBASE[fc]
                    hp = ph1ps.tile([P, OUT_FEATS], dt.float32, tag="hps")
                    for k in range(2):
                        nc.tensor.matmul(
                            out=hp[:],
                            lhsT=ft_sb[(fc, k)][:, tc_rel * P:(tc_rel + 1) * P],
                            rhs=w_sb[:, k, :],
                            start=(k == 0), stop=(k == 1),
                        )
                    h = ph1.tile([P, OUT_FEATS], dt.float32, tag="h")
                    nc.vector.tensor_copy(out=h[:], in_=hp[:])
                    m1 = ph1.tile([P, 8], dt.float32, tag="m1")
                    nc.vector.max(m1[:], h[:])
                    hneg = ph1.tile([P, OUT_FEATS], dt.float32, tag="hneg")
                    nc.vector.match_replace(out=hneg[:], in_to_replace=m1[:],
                                            in_values=h[:], imm_value=NEG_INF)
                    m2 = ph1.tile([P, 8], dt.float32, tag="m2")
                    nc.vector.max(m2[:], hneg[:])
                    mask = ph1.tile([P, OUT_FEATS], dt.float32, tag="mask")
                    nc.vector.tensor_tensor(
                        out=mask[:], in0=h[:],
                        in1=m2[:, 7:8].to_broadcast([P, OUT_FEATS]),
                        op=mybir.AluOpType.is_ge)
                    nc.vector.tensor_mul(out=h[:], in0=h[:], in1=mask[:])
                    nc.vector.tensor_tensor(
                        out=h[:], in0=h[:],
                        in1=scale_sb[:, t:t + 1].to_broadcast([P, OUT_FEATS]),
                        op=mybir.AluOpType.mult)
                    ttile = ph1.tile([P, PAIR], dt.bfloat16, tag="ttile")
                    hi32 = ph1.tile([P, OUT_FEATS], dt.float32, tag="hi32")
                    nc.scalar.activation(out=ttile[:, 0:OUT_FEATS], in_=h[:],
                                         func=mybir.ActivationFunctionType.Copy)
                    nc.scalar.activation(out=hi32[:], in_=ttile[:, 0:OUT_FEATS],
                                         func=mybir.ActivationFunctionType.Copy)
                    nc.vector.tensor_sub(out=hi32[:], in0=h[:], in1=hi32[:])
                    nc.vector.tensor_copy(out=ttile[:, OUT_FEATS:PAIR], in_=hi32[:])
                    nc.sync.dma_start(out=tableL[t * P:(t + 1) * P, :], in_=ttile[:])

            # ---- allgather table, one collective per shard-quarter so
            # phase-2 gathers can start before phase 1 fully drains ----
            for q in range(N_SLABS):
                nc.gpsimd.collective_compute(
                    "AllGather",
                    mybir.AluOpType.bypass,
                    replica_groups=[list(range(N_CORES))],
                    ins=[tableL[q * QROWS:(q + 1) * QROWS, :]],
                    outs=[tableQ[q][:]],
                )

            # ---- phase 2: edge aggregation ----
            phase2_stack = __import__("contextlib").ExitStack()
            accp = phase2_stack.enter_context(
                tc.tile_pool(name="accp", bufs=1, space="PSUM"))
            g_tiles = {}
            for ci, (s, j0, ct) in enumerate(calls):
                g = gp.tile([P, TILES_PER_CALL, PAIR], dt.bfloat16, tag="g")
                nc.gpsimd.dma_gather(
                    out_ap=g[:, :ct, :],
                    in_ap=tableQ[s][:],
                    idxs_ap=idx_sb[:, j0 * 8:(j0 + ct) * 8],
                    num_idxs=ct * P,
                    num_idxs_reg=ct * P,
                    elem_size=PAIR,
                    single_packet=False,
                    queue_num=ci % 4,
                )
                for t in range(ct):
                    g_tiles[j0 + t] = (g, t)

            # batched one-hot builds: one DVE op per 4 tiles
            SW = 4
            s_tiles = {}
            for j0 in range(0, ntiles, SW):
                jn = min(SW, ntiles - j0)
                s4 = sps.tile([P, SW, P], dt.bfloat16, tag="s")
                nc.vector.tensor_tensor(
                    out=s4[:, :jn, :],
                    in0=dstv_sb[:, j0:j0 + jn, :].to_broadcast([P, jn, P]),
                    in1=iota_sb[:].to_broadcast([P, jn, P]),
                    op=mybir.AluOpType.is_equal)
                for t in range(jn):
                    s_tiles[j0 + t] = (s4, t)

            cur_pass = -1
            acc = None
            ostage = None
            for j, (p, s, b, st, sp_) in enumerate(tile_meta):
                if p != cur_pass:
                    # one 2KB bank (512 fp32) per block so each block's
                    # matmul accumulation group owns its bank
                    acc = accp.tile([P, max(PASS_BLOCKS), 512], dt.float32,
                                    tag="acc")
                    ostage = outp.tile([P, max(PASS_BLOCKS), OUT_FEATS],
                                       dt.float32, tag="ostage")
                    cur_pass = p
                b_rel = b - PASS_BASE[p]
                s4_t, s4_i = s_tiles[j]
                s_t = s4_t[:, s4_i, :]
                g, gt = g_tiles[j]
                nc.tensor.matmul(
                    out=acc[:, b_rel, 0:PAIR],
                    lhsT=s_t[:],
                    rhs=g[:, gt, :],
                    start=bool(st), stop=bool(sp_),
                    skip_group_check=True,
                )
                if sp_:
                    nc.vector.tensor_add(out=ostage[:, b_rel, :],
                                         in0=acc[:, b_rel, 0:OUT_FEATS],
                                         in1=bias_sb[:])
                    nc.vector.tensor_add(out=ostage[:, b_rel, :],
                                         in0=ostage[:, b_rel, :],
                                         in1=acc[:, b_rel, OUT_FEATS:PAIR])
                    nc.sync.dma_start(out=out_d[b * P:(b + 1) * P, :],
                                      in_=ostage[:, b_rel, :])
            phase2_stack.close()

    nc.finalize()
    return nc


def kernel(feat, weight, bias, src, dst):
    feat = np.asarray(feat, dtype=np.float32)
    weight = np.asarray(weight, dtype=np.float32)
    bias = np.asarray(bias, dtype=np.float32)
    src = np.asarray(src)
    dst = np.asarray(dst)

    gidx_of, dstrel_of, counts, T = _inspect(src, dst)
    tile_meta, calls = _make_schedule(T)

    in_deg = np.bincount(dst, minlength=N_NODES).astype(np.float32)
    out_deg = np.bincount(src, minlength=N_NODES).astype(np.float32)
    ft = feat.T  # [256, 100000]
    iota = np.tile(np.arange(P, dtype=np.float32), (P, 1)).astype(ml_dtypes.bfloat16)

    in_maps = []
    for c in range(N_CORES):
        lo, hi = c * SHARD, (c + 1) * SHARD
        featT_c = np.zeros((IN_FEATS, SHARD_PAD), dtype=np.float32)
        featT_c[:, :SHARD] = ft[:, lo:hi]
        ideg_c = np.ones(SHARD_PAD, dtype=np.float32)
        odeg_c = np.ones(SHARD_PAD, dtype=np.float32)
        ideg_c[:SHARD] = in_deg[lo:hi]
        odeg_c[:SHARD] = out_deg[lo:hi]
        idx_wrapped, dstv = _per_core_streams(c, tile_meta, counts,
                                              gidx_of, dstrel_of)
        in_maps.append({
            "featT": featT_c,
            "w": weight,
            "biasb": np.tile(bias[None, :], (P, 1)).astype(np.float32),
            "idegw": ideg_c.reshape(N_BLOCKS, P).T.copy(),
            "odegw": odeg_c.reshape(N_BLOCKS, P).T.copy(),
            "iota": iota,
            "idxs": idx_wrapped,
            "dstv": dstv.astype(ml_dtypes.bfloat16),
        })

    nc = _build(tile_meta, calls)
    res = run_bass_kernel_spmd(nc, in_maps, list(range(N_CORES)),
                               trace=bool(os.environ.get("KERNEL_TRACE")))
    if os.environ.get("KERNEL_TRACE"):
        print(f"HW exec time: {res.exec_time_ns} ns")
    out = np.concatenate(
        [res.results[c]["out"][:SHARD] for c in range(N_CORES)], axis=0)
    return out.astype(np.float32)
